# revision 2
# baseline (speedup 1.0000x reference)
"""Multi-head causal attention (RoPE + per-head RMSNorm) on 8 TRN2 NeuronCores.

Reference computation (B=4, T=2048, C=1024, H=16, D=64):
    kqv = x @ W_kqv.T ; k,q,v = split(kqv) ; heads ; RoPE(q,k) ; RMSNorm(q,k)
    att = softmax(causal(q k^T / sqrt(D))) ; y = att v ; out = y @ W_proj.T

Sharding: core c -> batch b = c//2, head group g = c%2 (heads 8g..8g+8).
Each core computes a partial out[b] over its 8 heads' channels; host sums the
two partials per batch.

v4 changes vs v3 (driven by HW phase bisection: A+C = 173us, B = 380us vs
TimelineSim's 220us for B -- phase B eats the entire HW/sim gap):
  - causal-mask multiplies moved from GPSIMD (software Q7 ucode, slow and
    latency-heavy on HW) to DVE ([128,128] f16 SBUF ops run in the 4x mode).
  - score PSUM ring deepened to 3 single banks: scores(si+1) no longer
    anti-depend on exp(si) (the old 2-ring made every block's score matmul
    wait for the previous block's exp -> HW semaphore latency on the
    critical path 160 times).
  - kqv PSUM->SBUF staging copies moved from DVE to ACT (idle in phase A);
    DVE keeps the squares/rope/norm multiplies.
  - RMS stats packed into ONE psum bank per pair-half (q pairs at strips
    0/1, k at strips 2/3), halving stat-ln ACT work and freeing a bank.
  - output projection (phase C) for chunk t interleaved into phase B of
    chunk t+1 -- its PE work hides under B's ACT-bound stretches instead
    of tailing the kernel.
  - const DMAs reordered so wq/wk/x(0) land first: first matmul starts
    ~4us in instead of ~22us.
"""

import sys

import numpy as np

sys.path.insert(0, "/opt/trn_rl_repo")

B, T, C, H, D = 4, 2048, 1024, 16, 64
N_CORES = 8
HPC = H // 2  # heads per core: 8
TC = 512  # t-chunk (matmul free dim)
NTC = T // TC  # 4
NST = T // 128  # 16 s/t subtiles

_STATE: dict = {}


def _make_bacc():
    import concourse.mybir as mybir
    from concourse import bacc
    from concourse.hw_specs import get_activation_tables
    import bass_rust as _bass_rust

    AF = mybir.ActivationFunctionType
    SHARED = "natural_log_exp_and_others"

    class PinnedTableBacc(bacc.Bacc):
        """Bacc whose activation-table pass serves Exp/Ln only from the one
        set containing both, so the kernel needs a single table load."""

        def insert_act_table_loads(self):
            has_activation = any(
                isinstance(i, mybir.InstActivation)
                for b in self.main_func.blocks
                for i in b.instructions
            )
            if not has_activation:
                return
            tables = []
            for name, funcs in get_activation_tables(self.m.arch).items():
                if name != SHARED:
                    funcs = funcs - {AF.Exp, AF.Ln}
                tables.append((name, funcs))
            _bass_rust.insert_act_table_loads(self, tables)

    return PinnedTableBacc


def _build_nc(loop_n=None):
    import concourse.mybir as mybir
    from concourse.tile import TileContext
    from contextlib import ExitStack

    f16 = mybir.dt.float16
    f32 = mybir.dt.float32
    AF = mybir.ActivationFunctionType

    nc = _make_bacc()(
        "TRN2",
        target_bir_lowering=False,
        debug=False,
        num_devices=N_CORES,
    )

    xT = nc.dram_tensor("xT", [NTC, 128, 8, TC], f16, kind="ExternalInput")
    wqT = nc.dram_tensor("wqT", [128, 8, 512], f16, kind="ExternalInput")
    wkT = nc.dram_tensor("wkT", [128, 8, 512], f16, kind="ExternalInput")
    wvT = nc.dram_tensor("wvT", [128, 8, 512], f16, kind="ExternalInput")
    wpT = nc.dram_tensor("wpT", [128, 4, 1024], f16, kind="ExternalInput")
    cosd = nc.dram_tensor("cosd", [128, T], f16, kind="ExternalInput")
    sind = nc.dram_tensor("sind", [128, T], f16, kind="ExternalInput")
    maskd = nc.dram_tensor("maskd", [128, 128], f16, kind="ExternalInput")
    p2d = nc.dram_tensor("p2d", [128, 128], f16, kind="ExternalInput")
    ocd = nc.dram_tensor("ocd", [128, 2], f16, kind="ExternalInput")
    obd = nc.dram_tensor("obd", [128, 2, 128], f16, kind="ExternalInput")
    outd = nc.dram_tensor("out", [T, C], f32, kind="ExternalOutput")

    with TileContext(nc) as tc, ExitStack() as ctx:
        const = ctx.enter_context(tc.tile_pool(name="const", bufs=1))
        xpool = ctx.enter_context(tc.tile_pool(name="xp", bufs=2))
        persist = ctx.enter_context(tc.tile_pool(name="persist", bufs=1))
        work = ctx.enter_context(tc.tile_pool(name="work", bufs=3))
        attp = ctx.enter_context(tc.tile_pool(name="attp", bufs=6))
        outp = ctx.enter_context(tc.tile_pool(name="outp", bufs=2))
        psA = ctx.enter_context(tc.tile_pool(name="psA", bufs=2, space="PSUM"))
        psB = ctx.enter_context(tc.tile_pool(name="psB", bufs=2, space="PSUM"))
        psY = ctx.enter_context(tc.tile_pool(name="psY", bufs=2, space="PSUM"))

        # ---- constants, in first-use order (wq/wk feed the first matmuls;
        # wp is not needed until the first phase C, a full chunk later) ----
        wq_sb = const.tile([128, 8, 512], f16, tag="wq")
        nc.sync.dma_start(wq_sb, wqT[:, :, :])
        # chunk-0 x lands right behind wq so the first projection can start
        # ~6us in instead of behind the whole 7MB const load
        xt0 = xpool.tile([128, 8, TC], f16, tag="x", name="xt")
        nc.sync.dma_start(xt0, xT[0])
        wk_sb = const.tile([128, 8, 512], f16, tag="wk")
        nc.sync.dma_start(wk_sb, wkT[:, :, :])
        cos_sb = const.tile([128, T], f16, tag="cos")
        nc.sync.dma_start(cos_sb, cosd[:, :])
        sin_sb = const.tile([128, T], f16, tag="sin")
        nc.sync.dma_start(sin_sb, sind[:, :])
        oc_sb = const.tile([128, 2], f16, tag="oc")
        nc.sync.dma_start(oc_sb, ocd[:, :])
        p2_sb = const.tile([128, 128], f16, tag="p2")
        nc.sync.dma_start(p2_sb, p2d[:, :])
        ob_sb = const.tile([128, 2, 128], f16, tag="ob")
        nc.sync.dma_start(ob_sb, obd[:, :, :])
        wv_sb = const.tile([128, 8, 512], f16, tag="wv")
        nc.sync.dma_start(wv_sb, wvT[:, :, :])
        mask_sb = const.tile([128, 128], f16, tag="mask")
        nc.sync.dma_start(mask_sb, maskd[:, :])
        wp_sb = const.tile([128, 4, 1024], f16, tag="wp")
        nc.sync.dma_start(wp_sb, wpT[:, :, :])
        ones16 = const.tile([65, 64], f16, tag="ones16")
        nc.vector.memset(ones16, 1.0)

        # ---- persistent activations ----
        qT = [
            persist.tile([128, T], f16, tag=f"qT{p}", name=f"qT{p}")
            for p in range(4)
        ]
        kT = [
            persist.tile([128, T], f16, tag=f"kT{p}", name=f"kT{p}")
            for p in range(4)
        ]
        yT = [
            persist.tile([128, T], f16, tag=f"yT{p}", name=f"yT{p}")
            for p in range(4)
        ]
        # v (and the denominator ones-column) carry a 2^-12 scale so that
        # 1/denominator spans fp16; cancels exactly at the division.
        VSC = 2.0 ** -12
        v_sb = persist.tile([128, NST, HPC, 65], f16, tag="v")
        nc.vector.memset(v_sb[:, :, :, 64:65], VSC)

        def kqv_mm(ps, w_sb, p, xt):
            for ci in range(8):
                nc.tensor.matmul(
                    ps,
                    lhsT=w_sb[:, ci, p * 128 : (p + 1) * 128],
                    rhs=xt[:, ci, :],
                    start=(ci == 0),
                    stop=(ci == 7),
                )

        def rope_pe(raw, dst, tsl):
            """dst = raw*cos + rotate_half(raw)*sin; the rotate is a signed
            permutation matmul (DVE cannot permute partitions: TensorTensor
            requires equal base partitions for SBUF inputs, NCC_IBIR297)."""
            rot = psA.tile([128, TC], f32, tag="kqv")
            nc.tensor.matmul(rot, lhsT=p2_sb, rhs=raw, start=True, stop=True)
            qsh = work.tile([128, TC], f16, tag="qsh")
            nc.vector.tensor_mul(qsh, rot, sin_sb[:, tsl])
            t1 = work.tile([128, TC], f16, tag="t1")
            nc.vector.tensor_mul(t1, raw, cos_sb[:, tsl])
            nc.vector.tensor_add(dst, t1, qsh)

        def proj_rope(xt, p, tsl, stg, ro_q, ro_k):
            """q,k for head pair p: projection, squares summed by a tiny
            matmul into a transient psA slice, staged to SBUF (free-dim
            slot per (pair, q/k) -- PSUM stat banks are gone), RoPE."""
            ps_q = psA.tile([128, TC], f32, tag="kqv")
            kqv_mm(ps_q, wq_sb, p, xt)
            qraw = work.tile([128, TC], f16, tag="qraw")
            nc.vector.tensor_copy(qraw, ps_q)
            sq_q = work.tile([128, TC], f16, tag="sq_q", bufs=2)
            nc.vector.tensor_mul(sq_q, qraw, qraw)
            # RoPE preserves row norms -> sums of squares from pre-RoPE values
            st_q = psA.tile([128, TC], f32, tag="kqv")
            nc.tensor.matmul(
                st_q[0:2, :], lhsT=oc_sb, rhs=sq_q, start=True, stop=True,
            )
            nc.vector.tensor_copy(stg[0:2, p, 0, :], st_q[0:2, :])

            ps_k = psA.tile([128, TC], f32, tag="kqv")
            kqv_mm(ps_k, wk_sb, p, xt)
            kraw = work.tile([128, TC], f16, tag="kraw")
            nc.vector.tensor_copy(kraw, ps_k)
            sq_k = work.tile([128, TC], f16, tag="sq_k", bufs=2)
            nc.vector.tensor_mul(sq_k, kraw, kraw)
            st_k = psA.tile([128, TC], f32, tag="kqv")
            nc.tensor.matmul(
                st_k[0:2, :], lhsT=oc_sb, rhs=sq_k, start=True, stop=True,
            )
            nc.vector.tensor_copy(stg[0:2, p, 1, :], st_k[0:2, :])

            rope_pe(qraw, ro_q, tsl)
            rope_pe(kraw, ro_k, tsl)

        def stats_all(st8, stg):
            # repack the 4 pairs' [2, q/k, 512] stat rows to 32-aligned
            # partitions (SBUF->SBUF DMA moves partitions freely; engines
            # cannot), then ONE batched ln + exp over all 16 stat rows:
            # rr = exp(-0.5 ln ss). Rows between the stat rows are stale
            # SBUF; their ln/exp results are never read.
            pk = work.tile([128, 2, TC], f16, tag="pk", bufs=1)
            for p in range(4):
                nc.sync.dma_start(
                    pk[32 * p : 32 * p + 2, :, :], stg[0:2, p, :, :]
                )
            ln_t = work.tile([128, 2, TC], f32, tag="lnt", bufs=1)
            nc.scalar.activation(ln_t, pk, AF.Ln)
            rr = work.tile([128, 2, TC], f16, tag="rr", bufs=1)
            nc.scalar.activation(rr, ln_t, AF.Exp, scale=-0.5)
            st8["rr"] = rr

        def phase_a_units(tci):
            """Phase A for chunk tci as 7 units interleavable between the
            previous chunk's phase-B pairs."""
            tsl = slice(tci * TC, (tci + 1) * TC)
            st8 = {}

            def u_start():
                st8["xt"] = xpool.tile([128, 8, TC], f16, tag="x", name="xt")
                nc.sync.dma_start(st8["xt"], xT[tci])
                st8["stg"] = work.tile(
                    [2, 4, 2, TC], f16, tag="stg", name="stg", bufs=1
                )
                st8["ro_q"] = [
                    work.tile([128, TC], f16, tag=f"roq{p}", name=f"roq{p}")
                    for p in range(4)
                ]
                st8["ro_k"] = [
                    work.tile([128, TC], f16, tag=f"rok{p}", name=f"rok{p}")
                    for p in range(4)
                ]
                proj_rope(
                    st8["xt"], 0, tsl, st8["stg"],
                    st8["ro_q"][0], st8["ro_k"][0],
                )

            def u_pair1():
                proj_rope(
                    st8["xt"], 1, tsl, st8["stg"],
                    st8["ro_q"][1], st8["ro_k"][1],
                )

            def u_pair2():
                proj_rope(
                    st8["xt"], 2, tsl, st8["stg"],
                    st8["ro_q"][2], st8["ro_k"][2],
                )

            def u_pair3():
                proj_rope(
                    st8["xt"], 3, tsl, st8["stg"],
                    st8["ro_q"][3], st8["ro_k"][3],
                )
                stats_all(st8, st8["stg"])

            def u_v():
                for st in range(4):
                    pv = psA.tile([128, TC], f32, tag="kqv")
                    for ci in range(8):
                        nc.tensor.matmul(
                            pv,
                            lhsT=st8["xt"][:, ci, st * 128 : (st + 1) * 128],
                            rhs=wv_sb[:, ci, :],
                            start=(ci == 0),
                            stop=(ci == 7),
                        )
                    nc.vector.tensor_scalar_mul(
                        v_sb[:, tci * 4 + st, :, 0:64],
                        pv.rearrange("p (h d) -> p h d", h=HPC),
                        VSC,
                    )

            def u_norm(plo, phi):
                def f():
                    # qT/kT = ro * broadcast(8*w*rr)
                    for p in range(plo, phi):
                        m = 32 * p
                        rr = st8["rr"]
                        bc_q = psA.tile([128, TC], f32, tag="kqv", name="bc_q")
                        nc.tensor.matmul(
                            bc_q, lhsT=ob_sb[m : m + 2, 0, :],
                            rhs=rr[m : m + 2, 0, :], start=True, stop=True,
                            tile_position=(m, 0),
                        )
                        nc.vector.tensor_mul(qT[p][:, tsl], st8["ro_q"][p], bc_q)
                        bc_k = psA.tile([128, TC], f32, tag="kqv", name="bc_k")
                        nc.tensor.matmul(
                            bc_k, lhsT=ob_sb[m : m + 2, 1, :],
                            rhs=rr[m : m + 2, 1, :],
                            start=True, stop=True, tile_position=(m, 0),
                        )
                        nc.vector.tensor_mul(kT[p][:, tsl], st8["ro_k"][p], bc_k)
                return f

            return [
                u_start, u_pair1, u_pair2, u_pair3,
                u_v, u_norm(0, 2), u_norm(2, 4),
            ]

        def pair_b(tci, p):
            """Phase B for both heads of pair p: the two heads' score
            matmuls contract disjoint 64-row halves of the PE array, so
            issued back-to-back they run concurrently. Causal mask applied
            on DVE; interior key-blocks first, diagonal blocks last."""
            tsl = slice(tci * TC, (tci + 1) * TC)
            n_s = 4 * (tci + 1)
            ps_y0 = psY.tile([128, TC], f32, tag="y", name="ps_y0")
            ps_y1 = psY.tile([128, TC], f32, tag="y", name="ps_y1")
            sis = list(range(n_s))
            for idx, si in enumerate(sis):
                delta = si * 128 - tci * TC
                d = max(delta, 0)
                csl = slice(d, TC)
                ssl = slice(si * 128, (si + 1) * 128)
                qsl = slice(tci * TC + d, (tci + 1) * TC)
                # both heads' scores in one two-bank tile so the block's exp
                # is a single ACT instruction (the exp stream is the kernel's
                # hard floor -- ~870ns per [128,512] half on HW)
                ps_s = psB.tile([128, 2, TC], f32, tag="sc", name="ps_s")
                nc.tensor.matmul(
                    ps_s[:, 0, csl], lhsT=kT[p][0:64, ssl],
                    rhs=qT[p][0:64, qsl], start=True, stop=True,
                )
                nc.tensor.matmul(
                    ps_s[:, 1, csl], lhsT=kT[p][64:128, ssl],
                    rhs=qT[p][64:128, qsl], start=True, stop=True,
                )
                at = attp.tile([128, 2, TC], f16, tag="at", name="at")
                nc.scalar.activation(
                    at[:, :, csl], ps_s[:, :, csl], AF.Exp, scale=0.125
                )
                if delta >= 0:
                    # only the leading 128 columns of the valid range cross
                    # the diagonal; DVE runs these [128,128] f16 SBUF
                    # multiplies in its fast mode
                    for hl in range(2):
                        nc.vector.tensor_mul(
                            at[:, hl, d : d + 128], at[:, hl, d : d + 128],
                            mask_sb,
                        )
                nc.tensor.matmul(
                    ps_y0[0:65, csl], lhsT=v_sb[:, si, 2 * p, 0:65],
                    rhs=at[:, 0, csl], start=(idx == 0), stop=(idx == n_s - 1),
                )
                nc.tensor.matmul(
                    ps_y1[0:65, csl], lhsT=v_sb[:, si, 2 * p + 1, 0:65],
                    rhs=at[:, 1, csl], start=(idx == 0), stop=(idx == n_s - 1),
                )
            # softmax denominator: stage y to SBUF (frees the bank region),
            # fp16 reciprocal straight off PSUM row 64, broadcast down 64
            # partitions at full PE rate into the just-staged (hence free)
            # PSUM region, one multiply to fp16 yT.
            for hl, ps_y in ((0, ps_y0), (1, ps_y1)):
                ystg = work.tile([65, TC], f32, tag="ystg", bufs=2)
                nc.vector.tensor_copy(ystg[0:64, :], ps_y[0:64, :])
                recw = work.tile([65, TC], f16, tag="recw")
                with nc.allow_low_precision(reason="1/denom' in [6.6e-4,4096]"):
                    nc.vector.reciprocal(recw[64:65, :], ps_y[64:65, :])
                nc.tensor.matmul(
                    ps_y[0:64, :], lhsT=ones16[64:65, :], rhs=recw[64:65, :],
                    start=True, stop=True,
                )
                if hl == 0:
                    nc.vector.tensor_mul(
                        yT[p][0:64, tsl], ystg[0:64, :], ps_y[0:64, :]
                    )
                else:
                    y16 = work.tile([64, TC], f16, tag="y16")
                    nc.vector.tensor_mul(y16, ystg[0:64, :], ps_y[0:64, :])
                    nc.sync.dma_start(yT[p][64:128, tsl], y16)

        def phase_c_st(st):
            """Output projection for one 128-row t-slice (partials over this
            core's channels)."""
            for co in range(2):
                po = psA.tile([128, TC], f32, tag="kqv", name="po")
                for p in range(4):
                    nc.tensor.matmul(
                        po,
                        lhsT=yT[p][:, st * 128 : (st + 1) * 128],
                        rhs=wp_sb[:, p, co * 512 : (co + 1) * 512],
                        start=(p == 0),
                        stop=(p == 3),
                    )
                ot = outp.tile([128, TC], f32, tag="o")
                nc.vector.tensor_copy(ot, po)
                nc.sync.dma_start(
                    outd[st * 128 : (st + 1) * 128, co * 512 : (co + 1) * 512],
                    ot,
                )

        def body():
            for u in phase_a_units(0):
                u()
            for tci in range(NTC):
                nxt = phase_a_units(tci + 1) if tci + 1 < NTC else []
                # distribute next chunk's A units and the previous chunk's
                # C slices between this chunk's B pairs
                sched = {0: nxt[0:2], 1: nxt[2:4], 2: nxt[4:6], 3: nxt[6:7]}
                for pp in range(4):
                    pair_b(tci, pp)
                    for u in sched.get(pp, []):
                        u()
                    if tci > 0:
                        phase_c_st(4 * (tci - 1) + pp)
            for st in range(NST - 4, NST):
                phase_c_st(st)

        if loop_n is None:
            body()
        else:
            with tc.For_i(0, loop_n, 1):
                body()

    return nc


def _get_nc(loop_n=None):
    key = ("nc", loop_n)
    if key not in _STATE:
        nc = _build_nc(loop_n)
        nc.finalize()
        _STATE[key] = nc
    return _STATE[key]


def _rope_tables():
    inv_freq = 1.0 / (10000.0 ** (np.arange(0, D, 2, dtype=np.float64) / D))
    t_pos = np.arange(T, dtype=np.float64)
    freqs = t_pos[:, None] * inv_freq[None, :]  # [T, 32]
    f2 = np.concatenate([freqs, freqs], axis=-1)  # [T, 64]
    cosT = np.cos(f2).T.astype(np.float16)  # [64, T]
    sinT = np.sin(f2).T.astype(np.float16)
    cos2 = np.concatenate([cosT, cosT], axis=0)  # [128, T]
    sin2 = np.concatenate([sinT, sinT], axis=0)
    return np.ascontiguousarray(cos2), np.ascontiguousarray(sin2)


def _prep_inputs(x, W_kqv, W_proj, q_norm_w, k_norm_w):
    x = np.asarray(x, dtype=np.float32)
    W_kqv = np.asarray(W_kqv, dtype=np.float32)
    W_proj = np.asarray(W_proj, dtype=np.float32)
    q_norm_w = np.asarray(q_norm_w, dtype=np.float32)
    k_norm_w = np.asarray(k_norm_w, dtype=np.float32)

    cos2, sin2 = _rope_tables()

    # signed rotate-half permutation (per 64-dim head, stacked twice)
    P = np.zeros((64, 64), dtype=np.float16)
    for i in range(32):
        P[i, i + 32] = -1.0
        P[i + 32, i] = 1.0
    P2 = np.zeros((128, 128), dtype=np.float16)
    P2[0:64, 0:64] = P
    P2[64:128, 64:128] = P
    p2T = np.ascontiguousarray(P2.T)

    # single [128,128] triangular causal block (tj >= si keeps)
    si = np.arange(128)[:, None]
    tj = np.arange(128)[None, :]
    mask = np.ascontiguousarray((tj >= si).astype(np.float16))

    # columns 0/1 sum the two heads' squares
    oc = np.zeros((128, 2), dtype=np.float16)
    oc[0:64, 0] = 1.0
    oc[64:128, 1] = 1.0

    # broadcast weights with the 8 = sqrt(D) of 1/rms folded in: one 2-row
    # block per pair at partitions 32p, q/k along the middle dim
    ob = np.zeros((128, 2, 128), dtype=np.float16)
    for m in range(4):
        for qk, w in ((0, q_norm_w), (1, k_norm_w)):
            ob[32 * m + 0, qk, 0:64] = 8.0 * w
            ob[32 * m + 1, qk, 64:128] = 8.0 * w

    def wt_kqv(rows):
        # rows: [512, 1024] -> lhsT layout [128, 8, 512] fp16
        wT = rows.T.astype(np.float16)  # [1024, 512]
        return np.ascontiguousarray(wT.reshape(8, 128, 512).transpose(1, 0, 2))

    Wk, Wq, Wv = W_kqv[0:C], W_kqv[C : 2 * C], W_kqv[2 * C : 3 * C]

    in_maps = []
    for c in range(N_CORES):
        b, g = c // 2, c % 2
        rs = slice(512 * g, 512 * (g + 1))
        xTb = x[b].T.astype(np.float16)  # [C, T]
        xTr = np.ascontiguousarray(
            xTb.reshape(8, 128, NTC, TC).transpose(2, 1, 0, 3)
        )  # [NTC, 128, 8, TC]
        wp = W_proj[:, rs].T.astype(np.float16)  # [512, 1024]
        wpr = np.ascontiguousarray(wp.reshape(4, 128, 1024).transpose(1, 0, 2))
        in_maps.append(
            {
                "xT": xTr,
                "wqT": wt_kqv(Wq[rs]),
                "wkT": wt_kqv(Wk[rs]),
                "wvT": wt_kqv(Wv[rs]),
                "wpT": wpr,
                "cosd": cos2,
                "sind": sin2,
                "maskd": mask,
                "p2d": p2T,
                "ocd": oc,
                "obd": ob,
            }
        )
    return in_maps


def _get_runner(loop_n=None):
    """Build (once) a cached jitted SPMD runner mirroring
    bass2jax.run_bass_via_pjrt, so repeated calls reuse the compiled NEFF."""
    key = ("runner", loop_n)
    if key in _STATE:
        return _STATE[key]

    import jax
    import concourse.mybir as mybir
    from concourse import bass2jax
    from concourse.bass2jax import _bass_exec_p, partition_id_tensor
    from jax.experimental.shard_map import shard_map
    from jax.sharding import Mesh, NamedSharding, PartitionSpec

    bass2jax.install_neuronx_cc_hook()
    nc = _get_nc(loop_n)

    partition_name = nc.partition_id_tensor.name if nc.partition_id_tensor else None
    in_names, out_names, out_avals, zero_outs = [], [], [], []
    for alloc in nc.m.functions[0].allocations:
        if not isinstance(alloc, mybir.MemoryLocationSet):
            continue
        name = alloc.memorylocations[0].name
        if alloc.kind == "ExternalInput":
            if name != partition_name:
                in_names.append(name)
        elif alloc.kind == "ExternalOutput":
            shape = tuple(alloc.tensor_shape)
            dtype = mybir.dt.np(alloc.dtype)
            out_names.append(name)
            out_avals.append(jax.core.ShapedArray(shape, dtype))
            zero_outs.append(np.zeros(shape, dtype))
    n_params = len(in_names)
    all_names = in_names + out_names
    if partition_name is not None:
        all_names.append(partition_name)

    def _body(*args):
        operands = list(args)
        if partition_name is not None:
            operands.append(partition_id_tensor())
        outs = _bass_exec_p.bind(
            *operands,
            out_avals=tuple(out_avals),
            in_names=tuple(all_names),
            out_names=tuple(out_names),
            lowering_input_output_aliases=(),
            sim_require_finite=True,
            sim_require_nnan=True,
            nc=nc,
        )
        return tuple(outs)

    devices = jax.devices()[:N_CORES]
    mesh = Mesh(np.asarray(devices), ("core",))
    spec = PartitionSpec("core")
    n_outs = len(out_names)
    sharded = jax.jit(
        shard_map(
            _body,
            mesh=mesh,
            in_specs=(spec,) * (n_params + n_outs),
            out_specs=(spec,) * n_outs,
            check_rep=False,
        ),
        keep_unused=True,
    )
    sharding = NamedSharding(mesh, spec)
    zeros_dev = [
        jax.device_put(
            np.zeros((N_CORES * z.shape[0], *z.shape[1:]), z.dtype), sharding
        )
        for z in zero_outs
    ]
    runner = {
        "sharded": sharded,
        "in_names": in_names,
        "out_names": out_names,
        "out_avals": out_avals,
        "zeros_dev": zeros_dev,
        "sharding": sharding,
    }
    _STATE[key] = runner
    return runner


def _concat_inputs(in_maps, runner):
    return [
        np.concatenate([np.asarray(in_maps[c][n]) for c in range(N_CORES)], axis=0)
        for n in runner["in_names"]
    ]


def _execute(in_maps):
    """Returns list (per core) of {out_name: np.ndarray}."""
    runner = _get_runner()
    concat_in = _concat_inputs(in_maps, runner)
    out_arrs = runner["sharded"](*concat_in, *runner["zeros_dev"])
    return [
        {
            n: np.asarray(out_arrs[i]).reshape(
                N_CORES, *runner["out_avals"][i].shape
            )[c]
            for i, n in enumerate(runner["out_names"])
        }
        for c in range(N_CORES)
    ]


def _wall(runner, in_maps, iters):
    import time
    import jax

    concat_in = [
        jax.device_put(a, runner["sharding"])
        for a in _concat_inputs(in_maps, runner)
    ]
    args = (*concat_in, *runner["zeros_dev"])
    jax.block_until_ready(runner["sharded"](*args))  # warmup
    times = []
    for _ in range(iters):
        t0 = time.perf_counter()
        jax.block_until_ready(runner["sharded"](*args))
        times.append(time.perf_counter() - t0)
    times.sort()
    return times


def _timed(in_maps, iters=20, n_lo=1, n_hi=33):
    """Per-pass HW time via two device-side repeat counts: the dispatch/tunnel
    overhead cancels in the difference."""
    r_lo = _get_runner(None if n_lo == 1 else n_lo)
    r_hi = _get_runner(n_hi)
    t_lo = _wall(r_lo, in_maps, iters)
    t_hi = _wall(r_hi, in_maps, iters)
    k = max(3, iters // 4)
    lo = sum(t_lo[:k]) / k
    hi = sum(t_hi[:k]) / k
    per_pass = (hi - lo) / (n_hi - n_lo)
    return per_pass, lo, hi


def kernel(**inputs):
    in_maps = _prep_inputs(**inputs)
    res = _execute(in_maps)
    out = np.zeros((B, T, C), dtype=np.float32)
    for c in range(N_CORES):
        out[c // 2] += res[c]["out"]
    return out
